# revision 7
# baseline (speedup 1.0000x reference)
"""Two-layer GraphConv (DGL norm='both') on 8 Trainium2 NeuronCores.

Strategy (dst-sharded graph parallel, v2):
  - Nodes split into 8 contiguous shards of 12500; core c owns dst-shard c and
    the ~200k edges whose dst lands in it.
  - Layer tables are bf16. Host pre-scales x by norm_src, transposes, casts;
    P0 is 98 plain matmuls off a resident SBUF copy of xsT, writing the core's
    hW1 shard, with the 4 fragment AllGathers issued as their rows complete.
  - Gathers: one dma_gather per (8-tile group, window) with a STATIC count;
    padding indices point at row 0 and are killed by all-zero one-hot columns.
    Global chunk layout is group-major, window-major, tile-minor so each call
    covers one contiguous chunk range.
  - Segment-sum over dst on the TensorEngine: per 128-edge chunk a one-hot
    matrix O[e, dst_local] (is_equal of dst-local ids against an iota row) is
    matmul'd against the gathered rows, accumulating in PSUM per dst tile.
  - Layer-2 AllGathers are issued inside the layer-1 aggregation loop with a
    2-group lookahead past each fragment's last producing tile, so the CC
    stream starts while aggregation continues and the in-order GPSIMD queue
    never stalls long on the collective's input semaphores.

One SPMD program runs on all cores; per-core graph structure lives in the
input data. Chunk capacities per (tile, window) are the max over the 8 cores.
"""

import os
import numpy as np
import ml_dtypes

N_NODES = 100000
N_EDGES = 1600000
D = 128
NC = 8
P = 128
SHARD = N_NODES // NC            # 12500
TILES = (SHARD + P - 1) // P     # 98 dst tiles/core (last tile 84 valid rows)
SHARD_PAD = TILES * P            # 12544
# the table is fragmented: fragment k holds local rows [k*FR,(k+1)*FR) of every
# shard, rank-major ([NC*FR, D] per fragment). Fragments double as the int16
# gather windows (NC*FR = 25000 <= 32768) and let each AllGather overlap the
# gather-bound aggregation phase (range-based deps).
NW = 4
FR = SHARD // NW                 # 3125 local rows per fragment

T_GROUP = 8                      # dst tiles per gather call group
# max chunks (128 idx each) per dma_gather: large calls amortize the ~1us
# SWDGE fixed cost but must not overflow the per-engine descriptor ring
CALL_MAX_CH = int(os.environ.get("CCAS_MAXCH", "8"))
NQUEUES = 4

BF16 = ml_dtypes.bfloat16

_cache = {}


def _plan(src, dst):
    """Host-side graph partitioning -> structural plan + per-core data."""
    deg_out = np.bincount(src, minlength=N_NODES)
    deg_in = np.bincount(dst, minlength=N_NODES)
    norm_src = 1.0 / np.sqrt(np.maximum(deg_out, 1.0))
    norm_dst = 1.0 / np.sqrt(np.maximum(deg_in, 1.0))

    shard_of = dst // SHARD
    src_r = src // SHARD
    src_l = src % SHARD
    win_of = src_l // FR
    frag_row = src_r * FR + src_l % FR

    counts = np.zeros((NC, TILES, NW), np.int64)
    per_core = []
    for c in range(NC):
        m = shard_of == c
        es, ed, ew = frag_row[m], dst[m], win_of[m]
        dloc = ed - c * SHARD
        tl = dloc // P
        order = np.lexsort((es, ew, tl))
        es, ew, tl, dloc = es[order], ew[order], tl[order], dloc[order]
        np.add.at(counts[c], (tl, ew), 1)
        per_core.append((es, ew, tl, dloc))

    cap = counts.max(axis=0)                      # [TILES, NW]
    cap_ch = (-(-cap // P)).astype(np.int64)      # chunks per (tile, window)
    for t in range(TILES):
        if cap_ch[t].sum() == 0:
            cap_ch[t, 0] = 1

    groups = [list(range(g, min(g + T_GROUP, TILES)))
              for g in range(0, TILES, T_GROUP)]

    # global chunk layout: group-major, window-major, tile-minor
    chunk_pos = np.zeros((TILES, NW), np.int64)
    group_ch0 = []
    group_nch = []
    calls = []        # per group: list of (w, out_off_local, col0, piece_ch)
    pos = 0
    for g, gts in enumerate(groups):
        group_ch0.append(pos)
        gcalls = []
        for w in range(NW):
            sec0 = pos
            for t in gts:
                chunk_pos[t, w] = pos
                pos += int(cap_ch[t, w])
            sec_n = pos - sec0
            off = 0
            while off < sec_n:
                piece = min(sec_n - off, CALL_MAX_CH)
                gcalls.append((w, sec0 - group_ch0[g] + off,
                               (sec0 + off) * 8, piece))
                off += piece
        group_nch.append(pos - group_ch0[g])
        calls.append(gcalls)
    total_chunks = pos
    idx_cols = total_chunks * 8
    gch_max = max(group_nch)

    # per-core arrays
    idx_all = np.zeros((NC, 16, idx_cols), np.int16)
    dstl_all = np.full((NC, P, total_chunks), -1.0, BF16)
    for c in range(NC):
        es, ew, tl, dloc = per_core[c]
        cnt = counts[c]
        pos_e = 0
        for t in range(TILES):
            for w in range(NW):
                n = int(cnt[t, w])
                lo, hi = pos_e, pos_e + n
                pos_e += n
                nch = int(cap_ch[t, w])
                if nch == 0:
                    continue
                cp = int(chunk_pos[t, w])
                seg = np.zeros(nch * P, np.int16)
                seg[:n] = es[lo:hi].astype(np.int16)
                idx_all[c, :, cp * 8:(cp + nch) * 8] = \
                    seg.reshape(nch * 8, 16).T
                dl = np.full(nch * P, -1.0, BF16)
                if n:
                    dl[:n] = (dloc[lo:hi] % P).astype(BF16)
                dstl_all[c, :, cp:cp + nch] = dl.reshape(nch, P).T

    def tilemajor(v, c):
        out = np.ones((SHARD_PAD,), np.float32)
        out[:SHARD] = v[c * SHARD:(c + 1) * SHARD]
        return np.ascontiguousarray(out.reshape(TILES, P).T)

    ns_tm = np.stack([tilemajor(norm_src, c) for c in range(NC)])
    nd_tm = np.stack([tilemajor(norm_dst, c) for c in range(NC)])

    plan = dict(groups=groups, calls=calls,
                cap_ch=cap_ch.tolist(), chunk_pos=chunk_pos.tolist(),
                group_ch0=group_ch0, group_nch=group_nch, gch_max=gch_max,
                total_chunks=total_chunks, idx_cols=idx_cols)
    data = dict(idx_all=idx_all, dstl_all=dstl_all,
                ns_tm=ns_tm, nd_tm=nd_tm, norm_src=norm_src)
    return plan, data


def _build(plan, with_bias, use_bf16):
    import concourse.bass as bass
    import concourse.mybir as mybir
    import concourse.tile as tile
    from concourse import bacc
    from concourse.masks import make_identity

    f32 = mybir.dt.float32
    gdt = mybir.dt.bfloat16 if use_bf16 else f32

    groups = plan["groups"]
    calls = plan["calls"]
    cap_ch = plan["cap_ch"]
    chunk_pos = plan["chunk_pos"]
    group_ch0 = plan["group_ch0"]
    gch_max = plan["gch_max"]
    idx_cols = plan["idx_cols"]
    total_chunks = plan["total_chunks"]

    # last tile writing rows of fragment k (frag rows [k*FR,(k+1)*FR))
    frag_done_tile = [((k + 1) * FR - 1) // P for k in range(NW)]

    nc = bacc.Bacc("TRN2", target_bir_lowering=False, debug=False,
                   num_devices=NC, num_swdge_queues=NQUEUES)

    xsT_in = nc.dram_tensor("xsT_in", [D, SHARD_PAD], gdt, kind="ExternalInput")
    w1_in = nc.dram_tensor("w1_in", [D, D], gdt, kind="ExternalInput")
    w2_in = nc.dram_tensor("w2_in", [D, D], gdt, kind="ExternalInput")
    idx_in = nc.dram_tensor("idx_in", [P, idx_cols], mybir.dt.int16, kind="ExternalInput")
    dstl_in = nc.dram_tensor("dstl_in", [P, total_chunks], mybir.dt.bfloat16, kind="ExternalInput")
    nd_in = nc.dram_tensor("nd_in", [P, TILES], f32, kind="ExternalInput")
    nds_in = nc.dram_tensor("nds_in", [P, TILES], f32, kind="ExternalInput")
    if with_bias:
        ns_in = nc.dram_tensor("ns_in", [P, TILES], f32, kind="ExternalInput")
        b1_in = nc.dram_tensor("b1_in", [P, D], f32, kind="ExternalInput")
        b2_in = nc.dram_tensor("b2_in", [P, D], f32, kind="ExternalInput")
    y_out = nc.dram_tensor("y_out", [SHARD, D], f32, kind="ExternalOutput")

    ag2_in = nc.dram_tensor("ag2_in", [SHARD, D], gdt, kind="Internal")
    ag1_in = nc.dram_tensor("ag1_in", [SHARD, D], gdt, kind="Internal")
    hw1_frag = [nc.dram_tensor(f"hw1_frag{k}", [NC * FR, D], gdt, kind="Internal",
                               addr_space="Shared") for k in range(NW)]
    hw2_frag = [nc.dram_tensor(f"hw2_frag{k}", [NC * FR, D], gdt, kind="Internal",
                               addr_space="Shared") for k in range(NW)]

    RELU = mybir.ActivationFunctionType.Relu
    COPY = mybir.ActivationFunctionType.Copy

    def all_gather(src_t, dst_t, k):
        nc.gpsimd.collective_compute(
            "AllGather", mybir.AluOpType.bypass,
            replica_groups=[list(range(NC))],
            ins=[src_t[k * FR:(k + 1) * FR, :]], outs=[dst_t[k][:]])

    with tile.TileContext(nc) as tc:
        with (
            tc.tile_pool(name="const", bufs=1) as const,
            tc.tile_pool(name="xio", bufs=3) as xio,
            tc.tile_pool(name="gbuf", bufs=2) as gbuf,
            tc.tile_pool(name="obuf", bufs=6) as obuf,
            tc.tile_pool(name="ep", bufs=3) as ep,
            tc.tile_pool(name="ps_agg", bufs=3, space="PSUM") as ps_agg,
            tc.tile_pool(name="ps_tr", bufs=2, space="PSUM") as ps_tr,
            tc.tile_pool(name="ps_mm", bufs=2, space="PSUM") as ps_mm,
        ):
            # ---- constants ----
            idx_t = const.tile([P, idx_cols], mybir.dt.int16)
            nc.sync.dma_start(out=idx_t[:], in_=idx_in[:])
            dstl_t = const.tile([P, total_chunks], mybir.dt.bfloat16)
            nc.sync.dma_start(out=dstl_t[:], in_=dstl_in[:])
            nd_t = const.tile([P, TILES], f32)
            nc.sync.dma_start(out=nd_t[:], in_=nd_in[:])
            nds_t = const.tile([P, TILES], f32)
            nc.sync.dma_start(out=nds_t[:], in_=nds_in[:])
            w1_t = const.tile([D, D], gdt)
            nc.sync.dma_start(out=w1_t[:], in_=w1_in[:])
            w2_t = const.tile([D, D], gdt)
            nc.sync.dma_start(out=w2_t[:], in_=w2_in[:])
            xsT_t = const.tile([D, SHARD_PAD], gdt)
            nc.sync.dma_start(out=xsT_t[:], in_=xsT_in[:])
            if with_bias:
                ns_t = const.tile([P, TILES], f32)
                nc.sync.dma_start(out=ns_t[:], in_=ns_in[:])
                b1_t = const.tile([P, D], f32)
                nc.sync.dma_start(out=b1_t[:], in_=b1_in[:])
                b2_t = const.tile([P, D], f32)
                nc.sync.dma_start(out=b2_t[:], in_=b2_in[:])
            ident = const.tile([P, P], gdt)
            make_identity(nc, ident[:])
            iota_i = const.tile([P, P], mybir.dt.int32)
            nc.gpsimd.iota(iota_i[:], pattern=[[1, P]], base=0, channel_multiplier=0)
            iota_b = const.tile([P, P], mybir.dt.bfloat16)
            nc.vector.tensor_copy(out=iota_b[:], in_=iota_i[:])

            # static gather-count registers, one per distinct size
            sizes = {pc * P for gcalls in calls for (_w, _o, _c, pc) in gcalls}
            cnt_reg = {s: nc.gpsimd.to_reg(s) for s in sorted(sizes)}

            def dense_mm(lhsT, w_t, ag_dst, t):
                mm = ps_mm.tile([P, D], f32, space="PSUM", tag="mm")
                nc.tensor.matmul(mm[:], lhsT=lhsT, rhs=w_t[:], start=True, stop=True)
                hw_sb = xio.tile([P, D], gdt, tag="hw_sb")
                nc.scalar.activation(hw_sb[:], mm[:], COPY)
                rows = min(SHARD - t * P, P)
                nc.sync.dma_start(out=ag_dst[t * P:t * P + rows, :], in_=hw_sb[:rows, :])

            # ---- P0: xsT (preloaded) @ W1 -> ag1_in, AllGathers interleaved --
            for t in range(TILES):
                dense_mm(xsT_t[:, t * P:(t + 1) * P], w1_t, ag1_in, t)
                for k in range(NW):
                    if frag_done_tile[k] == t:
                        all_gather(ag1_in, hw1_frag, k)

            qn = [0]

            def agg_phase(frags, layer):
                # AG issue points: 2-group lookahead past the fragment's last tile
                ag_after_group = {}
                if layer == 1:
                    for k in range(NW):
                        gi = frag_done_tile[k] // T_GROUP + 2
                        ag_after_group.setdefault(min(gi, len(groups) - 1), []).append(k)
                kt_max = max(sum(cap_ch[t]) for t in range(TILES))
                for g, gts in enumerate(groups):
                    gc0 = group_ch0[g]
                    G = gbuf.tile([P, gch_max, D], gdt, tag="G")
                    for (w, out_off, col0, piece) in calls[g]:
                        nc.gpsimd.dma_gather(
                            G[:, out_off:out_off + piece, :],
                            frags[w][:],
                            idx_t[:, col0:col0 + piece * 8],
                            piece * P, cnt_reg[piece * P], D,
                            queue_num=qn[0] % NQUEUES)
                        qn[0] += 1
                    for t in gts:
                        kt = sum(cap_ch[t])
                        O = obuf.tile([P, kt_max, P], mybir.dt.bfloat16, tag="O")
                        oc = 0
                        for w in range(NW):
                            nch = cap_ch[t][w]
                            if nch == 0:
                                continue
                            cp = chunk_pos[t][w]
                            nc.vector.tensor_tensor(
                                out=O[:, oc:oc + nch, :],
                                in0=dstl_t[:, cp:cp + nch].unsqueeze(2).to_broadcast([P, nch, P]),
                                in1=iota_b[:].unsqueeze(1).to_broadcast([P, nch, P]),
                                op=mybir.AluOpType.is_equal)
                            oc += nch
                        agg = ps_agg.tile([P, D], f32, space="PSUM", tag="agg")
                        j = 0
                        for w in range(NW):
                            nch = cap_ch[t][w]
                            cp = chunk_pos[t][w]
                            for jj in range(nch):
                                nc.tensor.matmul(
                                    agg[:], lhsT=O[:, j, :],
                                    rhs=G[:, cp - gc0 + jj, :],
                                    start=(j == 0), stop=(j == kt - 1))
                                j += 1
                        if layer == 1:
                            t2 = ep.tile([P, D], gdt, tag="t2")
                            if with_bias:
                                z = ep.tile([P, D], f32, tag="z")
                                nc.vector.tensor_tensor(
                                    out=z[:], in0=agg[:],
                                    in1=nd_t[:, t:t + 1].to_broadcast([P, D]),
                                    op=mybir.AluOpType.mult)
                                nc.vector.tensor_add(out=z[:], in0=z[:], in1=b1_t[:])
                                nc.scalar.activation(t2[:], z[:], RELU,
                                                     scale=ns_t[:, t:t + 1])
                            else:
                                nc.scalar.activation(t2[:], agg[:], RELU,
                                                     scale=nds_t[:, t:t + 1])
                            tp = ps_tr.tile([P, P], gdt, space="PSUM", tag="tr")
                            nc.tensor.transpose(tp[:], t2[:], ident[:])
                            t2T = ep.tile([P, P], gdt, tag="t2T")
                            nc.scalar.activation(t2T[:], tp[:], COPY)
                            dense_mm(t2T[:], w2_t, ag2_in, t)
                        else:
                            y = ep.tile([P, D], f32, tag="y")
                            if with_bias:
                                z = ep.tile([P, D], f32, tag="z")
                                nc.vector.tensor_tensor(
                                    out=z[:], in0=agg[:],
                                    in1=nd_t[:, t:t + 1].to_broadcast([P, D]),
                                    op=mybir.AluOpType.mult)
                                nc.vector.tensor_add(out=z[:], in0=z[:], in1=b2_t[:])
                                nc.scalar.activation(y[:], z[:], RELU)
                            else:
                                nc.scalar.activation(y[:], agg[:], RELU,
                                                     scale=nd_t[:, t:t + 1])
                            rows = min(SHARD - t * P, P)
                            nc.sync.dma_start(out=y_out[t * P:t * P + rows, :],
                                              in_=y[:rows, :])
                    if layer == 1:
                        for k in ag_after_group.get(g, ()):
                            all_gather(ag2_in, hw2_frag, k)

            phases = int(os.environ.get("CCAS_PHASES", "5"))
            if phases >= 3:
                agg_phase(hw1_frag, layer=1)
            if phases >= 5:
                agg_phase(hw2_frag, layer=2)

    nc.compile()
    return nc


def kernel(x, W1, b1, W2, b2, src, dst):
    from concourse.bass_utils import run_bass_kernel_spmd

    src = np.asarray(src).astype(np.int64)
    dst = np.asarray(dst).astype(np.int64)
    x = np.asarray(x, dtype=np.float32)
    W1 = np.asarray(W1, dtype=np.float32)
    W2 = np.asarray(W2, dtype=np.float32)
    b1 = np.asarray(b1, dtype=np.float32)
    b2 = np.asarray(b2, dtype=np.float32)

    plan, data = _plan(src, dst)
    with_bias = bool(np.any(b1) or np.any(b2))
    use_bf16 = os.environ.get("CCAS_DT", "bf16") == "bf16"

    key = (with_bias, use_bf16, os.environ.get("CCAS_PHASES", "5"),
           repr(plan["calls"]), repr(plan["cap_ch"]))
    key = hash(key)
    if key not in _cache:
        _cache[key] = _build(plan, with_bias, use_bf16)
    nc = _cache[key]

    wdt = BF16 if use_bf16 else np.float32
    norm_src = data["norm_src"]
    in_maps = []
    for c in range(NC):
        xp = np.zeros((SHARD_PAD, D), np.float32)
        xp[:SHARD] = x[c * SHARD:(c + 1) * SHARD] \
            * norm_src[c * SHARD:(c + 1) * SHARD, None]
        m = dict(
            xsT_in=np.ascontiguousarray(xp.T).astype(wdt),
            w1_in=W1.astype(wdt),
            w2_in=W2.astype(wdt),
            idx_in=np.tile(data["idx_all"][c], (8, 1)),
            dstl_in=data["dstl_all"][c],
            nd_in=data["nd_tm"][c],
            nds_in=data["nd_tm"][c] * data["ns_tm"][c],
        )
        if with_bias:
            m["ns_in"] = data["ns_tm"][c]
            m["b1_in"] = np.broadcast_to(b1, (P, D)).astype(np.float32).copy()
            m["b2_in"] = np.broadcast_to(b2, (P, D)).astype(np.float32).copy()
        in_maps.append(m)

    prof_dir = os.environ.get("CCAS_PROFILE_DIR")
    if prof_dir:
        import sys, types
        if "antenv.axon_hooks" not in sys.modules:
            import antenv
            mod = types.ModuleType("antenv.axon_hooks")
            mod._hook = None
            mod.set_axon_ntff_profile_hook = lambda h: setattr(mod, "_hook", h)
            mod.get_axon_ntff_profile_hook = lambda: mod._hook
            sys.modules["antenv.axon_hooks"] = mod
            antenv.axon_hooks = mod
            from trn_agent_boot.trn_boot import _ntff_profile_via_ctypes
            mod.set_axon_ntff_profile_hook(
                _ntff_profile_via_ctypes("/opt/axon/libaxon_pjrt.so"))
        from antenv.axon_hooks import get_axon_ntff_profile_hook
        res = run_bass_kernel_spmd(nc, in_maps, core_ids=list(range(NC)))
        hook = get_axon_ntff_profile_hook()
        with hook(prof_dir, list(range(NC))):
            res = run_bass_kernel_spmd(nc, in_maps, core_ids=list(range(NC)))
    else:
        res = run_bass_kernel_spmd(nc, in_maps, core_ids=list(range(NC)))

    return np.concatenate([res.results[c]["y_out"] for c in range(NC)], axis=0)


# revision 13
# speedup vs baseline: 1.0943x; 1.0943x over previous
"""Two-layer GraphConv (DGL norm='both') on 8 Trainium2 NeuronCores.

Strategy (dst-sharded graph parallel, v2):
  - Nodes split into 8 contiguous shards of 12500; core c owns dst-shard c and
    the ~200k edges whose dst lands in it.
  - Layer tables are bf16. Host pre-scales x by norm_src, transposes, casts;
    P0 is 98 plain matmuls off a resident SBUF copy of xsT, writing the core's
    hW1 shard, with the 4 fragment AllGathers issued as their rows complete.
  - Gathers: one dma_gather per (8-tile group, window) with a STATIC count;
    padding indices point at row 0 and are killed by all-zero one-hot columns.
    Global chunk layout is group-major, window-major, tile-minor so each call
    covers one contiguous chunk range.
  - Segment-sum over dst on the TensorEngine: per 128-edge chunk a one-hot
    matrix O[e, dst_local] (is_equal of dst-local ids against an iota row) is
    matmul'd against the gathered rows, accumulating in PSUM per dst tile.
  - Layer-2 AllGathers are issued inside the layer-1 aggregation loop with a
    2-group lookahead past each fragment's last producing tile, so the CC
    stream starts while aggregation continues and the in-order GPSIMD queue
    never stalls long on the collective's input semaphores.

One SPMD program runs on all cores; per-core graph structure lives in the
input data. Chunk capacities per (tile, window) are the max over the 8 cores.
"""

import os
import numpy as np
import ml_dtypes

N_NODES = 100000
N_EDGES = 1600000
D = 128
NC = 8
P = 128
SHARD = N_NODES // NC            # 12500
TILES = (SHARD + P - 1) // P     # 98 dst tiles/core (last tile 84 valid rows)
SHARD_PAD = TILES * P            # 12544
# the table is fragmented: fragment k holds local rows [k*FR,(k+1)*FR) of every
# shard, rank-major ([NC*FR, D] per fragment). Fragments double as the int16
# gather windows (NC*FR = 25000 <= 32768) and let each AllGather overlap the
# gather-bound aggregation phase (range-based deps).
NW = 4
FR = SHARD // NW                 # 3125 local rows per fragment

T_GROUP = 4                      # dst tiles per gather call group
# max chunks (128 idx each) per dma_gather: large calls amortize the ~1us
# SWDGE fixed cost but must fit the per-queue descriptor ring (SCRATCH/16)
CALL_MAX_CH = int(os.environ.get("CCAS_MAXCH", "10"))
SCRATCH = max(16384, CALL_MAX_CH * 128 * 16)
NQUEUES = 4
# gathered rows are stored twice ([row | row], 512B): sub-512B DMA
# descriptors drain at half rate, so duplicated bf16 rows gather as fast as
# f32 while keeping the PE on the 4x-fast bf16 path
DUP = 2

BF16 = ml_dtypes.bfloat16

_cache = {}


def _plan(src, dst):
    """Host-side graph partitioning -> structural plan + per-core data."""
    deg_out = np.bincount(src, minlength=N_NODES)
    deg_in = np.bincount(dst, minlength=N_NODES)
    norm_src = 1.0 / np.sqrt(np.maximum(deg_out, 1.0))
    norm_dst = 1.0 / np.sqrt(np.maximum(deg_in, 1.0))

    shard_of = dst // SHARD
    src_r = src // SHARD
    src_l = src % SHARD
    win_of = src_l // FR
    frag_row = src_r * FR + src_l % FR

    counts = np.zeros((NC, TILES, NW), np.int64)
    per_core = []
    for c in range(NC):
        m = shard_of == c
        es, ed, ew = frag_row[m], dst[m], win_of[m]
        dloc = ed - c * SHARD
        tl = dloc // P
        order = np.lexsort((es, ew, tl))
        es, ew, tl, dloc = es[order], ew[order], tl[order], dloc[order]
        np.add.at(counts[c], (tl, ew), 1)
        per_core.append((es, ew, tl, dloc))

    cap = counts.max(axis=0)                      # [TILES, NW]
    cap_ch = (-(-cap // P)).astype(np.int64)      # chunks per (tile, window)
    for t in range(TILES):
        if cap_ch[t].sum() == 0:
            cap_ch[t, 0] = 1

    groups = [list(range(g, min(g + T_GROUP, TILES)))
              for g in range(0, TILES, T_GROUP)]

    # global chunk layout: group-major, window-major, tile-minor
    chunk_pos = np.zeros((TILES, NW), np.int64)
    group_ch0 = []
    group_nch = []
    calls = []        # per group: list of (w, out_off_local, col0, piece_ch)
    pos = 0
    for g, gts in enumerate(groups):
        group_ch0.append(pos)
        gcalls = []
        for w in range(NW):
            sec0 = pos
            for t in gts:
                chunk_pos[t, w] = pos
                pos += int(cap_ch[t, w])
            sec_n = pos - sec0
            off = 0
            while off < sec_n:
                piece = min(sec_n - off, CALL_MAX_CH)
                gcalls.append((w, sec0 - group_ch0[g] + off,
                               (sec0 + off) * 8, piece))
                off += piece
        group_nch.append(pos - group_ch0[g])
        calls.append(gcalls)
    total_chunks = pos
    idx_cols = total_chunks * 8
    gch_max = max(group_nch)

    # per-core arrays
    idx_all = np.zeros((NC, 16, idx_cols), np.int16)
    dstl_all = np.full((NC, P, total_chunks), -1.0, BF16)
    for c in range(NC):
        es, ew, tl, dloc = per_core[c]
        cnt = counts[c]
        pos_e = 0
        for t in range(TILES):
            for w in range(NW):
                n = int(cnt[t, w])
                lo, hi = pos_e, pos_e + n
                pos_e += n
                nch = int(cap_ch[t, w])
                if nch == 0:
                    continue
                cp = int(chunk_pos[t, w])
                seg = np.zeros(nch * P, np.int16)
                seg[:n] = es[lo:hi].astype(np.int16)
                idx_all[c, :, cp * 8:(cp + nch) * 8] = \
                    seg.reshape(nch * 8, 16).T
                dl = np.full(nch * P, -1.0, BF16)
                if n:
                    dl[:n] = (dloc[lo:hi] % P).astype(BF16)
                dstl_all[c, :, cp:cp + nch] = dl.reshape(nch, P).T

    def tilemajor(v, c):
        out = np.ones((SHARD_PAD,), np.float32)
        out[:SHARD] = v[c * SHARD:(c + 1) * SHARD]
        return np.ascontiguousarray(out.reshape(TILES, P).T)

    ns_tm = np.stack([tilemajor(norm_src, c) for c in range(NC)])
    nd_tm = np.stack([tilemajor(norm_dst, c) for c in range(NC)])

    plan = dict(groups=groups, calls=calls,
                cap_ch=cap_ch.tolist(), chunk_pos=chunk_pos.tolist(),
                group_ch0=group_ch0, group_nch=group_nch, gch_max=gch_max,
                total_chunks=total_chunks, idx_cols=idx_cols)
    data = dict(idx_all=idx_all, dstl_all=dstl_all,
                ns_tm=ns_tm, nd_tm=nd_tm, norm_src=norm_src)
    return plan, data


def _build(plan, with_bias, use_bf16):
    import concourse.bass as bass
    import concourse.mybir as mybir
    import concourse.tile as tile
    from concourse import bacc
    from concourse.masks import make_identity

    f32 = mybir.dt.float32
    gdt = mybir.dt.bfloat16 if use_bf16 else f32

    groups = plan["groups"]
    calls = plan["calls"]
    cap_ch = plan["cap_ch"]
    chunk_pos = plan["chunk_pos"]
    group_ch0 = plan["group_ch0"]
    gch_max = plan["gch_max"]
    idx_cols = plan["idx_cols"]
    total_chunks = plan["total_chunks"]

    # last tile writing rows of fragment k (frag rows [k*FR,(k+1)*FR))
    frag_done_tile = [((k + 1) * FR - 1) // P for k in range(NW)]

    nc = bacc.Bacc("TRN2", target_bir_lowering=False, debug=False,
                   num_devices=NC, num_swdge_queues=NQUEUES,
                   dynamic_dma_scratch_size=SCRATCH)

    xsT_in = nc.dram_tensor("xsT_in", [D, SHARD_PAD], gdt, kind="ExternalInput")
    w1_in = nc.dram_tensor("w1_in", [D, D], gdt, kind="ExternalInput")
    w2_in = nc.dram_tensor("w2_in", [D, D], gdt, kind="ExternalInput")
    idx_in = nc.dram_tensor("idx_in", [P, idx_cols], mybir.dt.int16, kind="ExternalInput")
    dstl_in = nc.dram_tensor("dstl_in", [P, total_chunks], mybir.dt.bfloat16, kind="ExternalInput")
    nd_in = nc.dram_tensor("nd_in", [P, TILES], f32, kind="ExternalInput")
    nds_in = nc.dram_tensor("nds_in", [P, TILES], f32, kind="ExternalInput")
    if with_bias:
        ns_in = nc.dram_tensor("ns_in", [P, TILES], f32, kind="ExternalInput")
        b1_in = nc.dram_tensor("b1_in", [P, D], f32, kind="ExternalInput")
        b2_in = nc.dram_tensor("b2_in", [P, D], f32, kind="ExternalInput")
    y_out = nc.dram_tensor("y_out", [SHARD, D], f32, kind="ExternalOutput")

    ag2_in = nc.dram_tensor("ag2_in", [SHARD, DUP * D], gdt, kind="Internal")
    ag1_in = nc.dram_tensor("ag1_in", [SHARD, DUP * D], gdt, kind="Internal")
    hw1_frag = [nc.dram_tensor(f"hw1_frag{k}", [NC * FR, DUP * D], gdt, kind="Internal",
                               addr_space="Shared") for k in range(NW)]
    hw2_frag = [nc.dram_tensor(f"hw2_frag{k}", [NC * FR, DUP * D], gdt, kind="Internal",
                               addr_space="Shared") for k in range(NW)]

    RELU = mybir.ActivationFunctionType.Relu
    COPY = mybir.ActivationFunctionType.Copy

    def all_gather(src_t, dst_t, k):
        nc.gpsimd.collective_compute(
            "AllGather", mybir.AluOpType.bypass,
            replica_groups=[list(range(NC))],
            ins=[src_t[k * FR:(k + 1) * FR, :]], outs=[dst_t[k][:]])

    with tile.TileContext(nc) as tc:
        with (
            tc.tile_pool(name="const", bufs=1) as const,
            tc.tile_pool(name="xio", bufs=3) as xio,
            tc.tile_pool(name="gbuf", bufs=2) as gbuf,
            tc.tile_pool(name="obuf", bufs=6) as obuf,
            tc.tile_pool(name="ep", bufs=3) as ep,
            tc.tile_pool(name="ps_agg", bufs=3, space="PSUM") as ps_agg,
            tc.tile_pool(name="ps_tr", bufs=2, space="PSUM") as ps_tr,
            tc.tile_pool(name="ps_mm", bufs=2, space="PSUM") as ps_mm,
        ):
            # ---- constants ----
            idx_t = const.tile([P, idx_cols], mybir.dt.int16)
            nc.sync.dma_start(out=idx_t[:], in_=idx_in[:])
            dstl_t = const.tile([P, total_chunks], mybir.dt.bfloat16)
            nc.sync.dma_start(out=dstl_t[:], in_=dstl_in[:])
            nd_t = const.tile([P, TILES], f32)
            nc.sync.dma_start(out=nd_t[:], in_=nd_in[:])
            nds_t = const.tile([P, TILES], f32)
            nc.sync.dma_start(out=nds_t[:], in_=nds_in[:])
            w1_t = const.tile([D, D], gdt)
            nc.sync.dma_start(out=w1_t[:], in_=w1_in[:])
            w2_t = const.tile([D, D], gdt)
            nc.sync.dma_start(out=w2_t[:], in_=w2_in[:])
            xsT_t = const.tile([D, SHARD_PAD], gdt)
            nc.sync.dma_start(out=xsT_t[:], in_=xsT_in[:])
            if with_bias:
                ns_t = const.tile([P, TILES], f32)
                nc.sync.dma_start(out=ns_t[:], in_=ns_in[:])
                b1_t = const.tile([P, D], f32)
                nc.sync.dma_start(out=b1_t[:], in_=b1_in[:])
                b2_t = const.tile([P, D], f32)
                nc.sync.dma_start(out=b2_t[:], in_=b2_in[:])
            ident = const.tile([P, P], gdt)
            make_identity(nc, ident[:])
            iota_i = const.tile([P, P], mybir.dt.int32)
            nc.gpsimd.iota(iota_i[:], pattern=[[1, P]], base=0, channel_multiplier=0)
            iota_b = const.tile([P, P], mybir.dt.bfloat16)
            nc.vector.tensor_copy(out=iota_b[:], in_=iota_i[:])

            # static gather-count registers, one per distinct size
            sizes = {pc * P for gcalls in calls for (_w, _o, _c, pc) in gcalls}
            cnt_reg = {s: nc.gpsimd.to_reg(s) for s in sorted(sizes)}

            def dense_mm(lhsT, w_t, ag_dst, t):
                mm = ps_mm.tile([P, D], f32, space="PSUM", tag="mm")
                nc.tensor.matmul(mm[:], lhsT=lhsT, rhs=w_t[:], start=True, stop=True)
                hw_sb = xio.tile([P, DUP * D], gdt, tag="hw_sb")
                for r in range(DUP):
                    nc.scalar.activation(hw_sb[:, r * D:(r + 1) * D], mm[:], COPY)
                rows = min(SHARD - t * P, P)
                nc.sync.dma_start(out=ag_dst[t * P:t * P + rows, :], in_=hw_sb[:rows, :])

            # ---- P0: xsT (preloaded) @ W1 -> ag1_in, AllGathers interleaved --
            for t in range(TILES):
                dense_mm(xsT_t[:, t * P:(t + 1) * P], w1_t, ag1_in, t)
                for k in range(NW):
                    if frag_done_tile[k] == t:
                        all_gather(ag1_in, hw1_frag, k)

            qn = [0]

            def agg_phase(frags, layer):
                # AG issue points: 2-group lookahead past the fragment's last tile
                ag_after_group = {}
                if layer == 1:
                    for k in range(NW):
                        gi = frag_done_tile[k] // T_GROUP + 2
                        ag_after_group.setdefault(min(gi, len(groups) - 1), []).append(k)
                kt_max = max(sum(cap_ch[t]) for t in range(TILES))
                for g, gts in enumerate(groups):
                    gc0 = group_ch0[g]
                    G = gbuf.tile([P, gch_max, DUP * D], gdt, tag="G")
                    for (w, out_off, col0, piece) in calls[g]:
                        nc.gpsimd.dma_gather(
                            G[:, out_off:out_off + piece, :],
                            frags[w][:],
                            idx_t[:, col0:col0 + piece * 8],
                            piece * P, cnt_reg[piece * P], DUP * D,
                            queue_num=qn[0] % NQUEUES)
                        qn[0] += 1
                    for t in gts:
                        kt = sum(cap_ch[t])
                        O = obuf.tile([P, kt_max, P], mybir.dt.bfloat16, tag="O")
                        oc = 0
                        for w in range(NW):
                            nch = cap_ch[t][w]
                            if nch == 0:
                                continue
                            cp = chunk_pos[t][w]
                            nc.vector.tensor_tensor(
                                out=O[:, oc:oc + nch, :],
                                in0=dstl_t[:, cp:cp + nch].unsqueeze(2).to_broadcast([P, nch, P]),
                                in1=iota_b[:].unsqueeze(1).to_broadcast([P, nch, P]),
                                op=mybir.AluOpType.is_equal)
                            oc += nch
                        agg = ps_agg.tile([P, D], f32, space="PSUM", tag="agg")
                        j = 0
                        for w in range(NW):
                            nch = cap_ch[t][w]
                            cp = chunk_pos[t][w]
                            for jj in range(nch):
                                nc.tensor.matmul(
                                    agg[:], lhsT=O[:, j, :],
                                    rhs=G[:, cp - gc0 + jj, 0:D],
                                    start=(j == 0), stop=(j == kt - 1))
                                j += 1
                        if layer == 1:
                            t2 = ep.tile([P, D], gdt, tag="t2")
                            if with_bias:
                                z = ep.tile([P, D], f32, tag="z")
                                nc.vector.tensor_tensor(
                                    out=z[:], in0=agg[:],
                                    in1=nd_t[:, t:t + 1].to_broadcast([P, D]),
                                    op=mybir.AluOpType.mult)
                                nc.vector.tensor_add(out=z[:], in0=z[:], in1=b1_t[:])
                                nc.scalar.activation(t2[:], z[:], RELU,
                                                     scale=ns_t[:, t:t + 1])
                            else:
                                nc.scalar.activation(t2[:], agg[:], RELU,
                                                     scale=nds_t[:, t:t + 1])
                            tp = ps_tr.tile([P, P], gdt, space="PSUM", tag="tr")
                            nc.tensor.transpose(tp[:], t2[:], ident[:])
                            t2T = ep.tile([P, P], gdt, tag="t2T")
                            nc.scalar.activation(t2T[:], tp[:], COPY)
                            dense_mm(t2T[:], w2_t, ag2_in, t)
                        else:
                            y = ep.tile([P, D], f32, tag="y")
                            if with_bias:
                                z = ep.tile([P, D], f32, tag="z")
                                nc.vector.tensor_tensor(
                                    out=z[:], in0=agg[:],
                                    in1=nd_t[:, t:t + 1].to_broadcast([P, D]),
                                    op=mybir.AluOpType.mult)
                                nc.vector.tensor_add(out=z[:], in0=z[:], in1=b2_t[:])
                                nc.scalar.activation(y[:], z[:], RELU)
                            else:
                                nc.scalar.activation(y[:], agg[:], RELU,
                                                     scale=nd_t[:, t:t + 1])
                            rows = min(SHARD - t * P, P)
                            nc.sync.dma_start(out=y_out[t * P:t * P + rows, :],
                                              in_=y[:rows, :])
                    if layer == 1:
                        for k in ag_after_group.get(g, ()):
                            all_gather(ag2_in, hw2_frag, k)

            phases = int(os.environ.get("CCAS_PHASES", "5"))
            if phases >= 3:
                agg_phase(hw1_frag, layer=1)
            if phases >= 5:
                agg_phase(hw2_frag, layer=2)

    nc.compile()
    return nc


def kernel(x, W1, b1, W2, b2, src, dst):
    from concourse.bass_utils import run_bass_kernel_spmd

    src = np.asarray(src).astype(np.int64)
    dst = np.asarray(dst).astype(np.int64)
    x = np.asarray(x, dtype=np.float32)
    W1 = np.asarray(W1, dtype=np.float32)
    W2 = np.asarray(W2, dtype=np.float32)
    b1 = np.asarray(b1, dtype=np.float32)
    b2 = np.asarray(b2, dtype=np.float32)

    plan, data = _plan(src, dst)
    with_bias = bool(np.any(b1) or np.any(b2))
    use_bf16 = os.environ.get("CCAS_DT", "bf16") == "bf16"

    key = (with_bias, use_bf16, os.environ.get("CCAS_PHASES", "5"),
           repr(plan["calls"]), repr(plan["cap_ch"]))
    key = hash(key)
    if key not in _cache:
        _cache[key] = _build(plan, with_bias, use_bf16)
    nc = _cache[key]

    wdt = BF16 if use_bf16 else np.float32
    norm_src = data["norm_src"]
    in_maps = []
    for c in range(NC):
        xp = np.zeros((SHARD_PAD, D), np.float32)
        xp[:SHARD] = x[c * SHARD:(c + 1) * SHARD] \
            * norm_src[c * SHARD:(c + 1) * SHARD, None]
        m = dict(
            xsT_in=np.ascontiguousarray(xp.T).astype(wdt),
            w1_in=W1.astype(wdt),
            w2_in=W2.astype(wdt),
            idx_in=np.tile(data["idx_all"][c], (8, 1)),
            dstl_in=data["dstl_all"][c],
            nd_in=data["nd_tm"][c],
            nds_in=data["nd_tm"][c] * data["ns_tm"][c],
        )
        if with_bias:
            m["ns_in"] = data["ns_tm"][c]
            m["b1_in"] = np.broadcast_to(b1, (P, D)).astype(np.float32).copy()
            m["b2_in"] = np.broadcast_to(b2, (P, D)).astype(np.float32).copy()
        in_maps.append(m)

    prof_dir = os.environ.get("CCAS_PROFILE_DIR")
    if prof_dir:
        import sys, types
        if "antenv.axon_hooks" not in sys.modules:
            import antenv
            mod = types.ModuleType("antenv.axon_hooks")
            mod._hook = None
            mod.set_axon_ntff_profile_hook = lambda h: setattr(mod, "_hook", h)
            mod.get_axon_ntff_profile_hook = lambda: mod._hook
            sys.modules["antenv.axon_hooks"] = mod
            antenv.axon_hooks = mod
            from trn_agent_boot.trn_boot import _ntff_profile_via_ctypes
            mod.set_axon_ntff_profile_hook(
                _ntff_profile_via_ctypes("/opt/axon/libaxon_pjrt.so"))
        from antenv.axon_hooks import get_axon_ntff_profile_hook
        res = run_bass_kernel_spmd(nc, in_maps, core_ids=list(range(NC)))
        hook = get_axon_ntff_profile_hook()
        with hook(prof_dir, list(range(NC))):
            res = run_bass_kernel_spmd(nc, in_maps, core_ids=list(range(NC)))
    else:
        res = run_bass_kernel_spmd(nc, in_maps, core_ids=list(range(NC)))

    return np.concatenate([res.results[c]["y_out"] for c in range(NC)], axis=0)
